# revision 28
# baseline (speedup 1.0000x reference)
"""Trainium2 Bass kernel for nn_CrossAttentionPro (chained cross-attention).

Sharding: 8 cores = data-parallel over B (2) x head-parallel (4 head-pairs).
Each core computes, for one batch b and heads (2*hp, 2*hp+1), the full
chained cross-attention restricted to its heads, with the T x T chain
collapsed algebraically: chained = qx G kx^T with G = ky^T qy.

v2 pipeline design (vs v1):
- score matmuls (K=64) for the two heads issued to distinct PE row groups
  (base partitions 0/64) so they run concurrently.
- exp split across two engines: ScalarE exact exp and a VectorE
  Schraudolph exp2 bit-trick (i16 = A*x + B bitcast to bf16, ~2% rms err)
  so the ~12.6M exps/core are not serialized on one engine.
- cval matmuls software-pipelined one block behind the score matmuls so
  the PE never waits on the activation engines.
- softmax denominators via an appended ones-column in the v-aug slabs
  (both heads interleaved in one aug tile, scatter-filled by one strided
  copy per 8 transposed blocks).
- normalization (reciprocal-broadcast-mult-sub) mostly on GpSimd; final
  tail combo on DVE to shorten the critical path.
- x-side projections (kx/vx/wx) and the tci=0 output projection are
  injected into the attention block loops to fill PE slack.
Host sums the 4 head-pair partials per batch and adds proj_b.
"""

import math
import numpy as np

B, T, MM, C, H = 2, 2048, 1024, 512, 8
D = 64
NC = 8
NMB = MM // 128  # 8 m-blocks
NSB = T // 128  # 16 s-blocks
_kernels = {}

# Schraudolph bf16 exp2 constants: i16 = round((x*log2e)*128 + BIAS)
_LOG2E = 1.4426950408889634
_SCH_A = 128.0 * _LOG2E
_SCH_B = 16248.58


def _install_ntff_hook():
    """Bridge antenv.axon_hooks for NTFF profiling (missing in this image)."""
    import contextlib, ctypes, sys, types

    if "antenv.axon_hooks" in sys.modules:
        return
    try:
        import antenv
    except ImportError:
        return

    def _make_hook():
        try:
            lib = ctypes.CDLL("/opt/axon/libaxon_pjrt.so")
        except OSError:
            return None
        if not hasattr(lib, "axon_start_nrt_profile"):
            return None
        lib.axon_start_nrt_profile.argtypes = [
            ctypes.POINTER(ctypes.c_int64),
            ctypes.c_size_t,
        ]
        lib.axon_start_nrt_profile.restype = ctypes.c_int64
        lib.axon_stop_nrt_profile.argtypes = [ctypes.c_char_p]
        lib.axon_stop_nrt_profile.restype = ctypes.c_int64

        @contextlib.contextmanager
        def _hook(output_dir, device_ids):
            import jax

            jax.devices()
            if device_ids:
                ids = (ctypes.c_int64 * len(device_ids))(*device_ids)
                rc = lib.axon_start_nrt_profile(ids, len(device_ids))
            else:
                rc = lib.axon_start_nrt_profile(None, 0)
            if rc != 0:
                raise RuntimeError(f"axon_start_nrt_profile rc={rc}")
            try:
                yield
            finally:
                n = lib.axon_stop_nrt_profile(str(output_dir).encode())
                if n < 0:
                    raise RuntimeError(f"axon_stop_nrt_profile rc={n}")

        return _hook

    m = types.ModuleType("antenv.axon_hooks")
    m._hook = _make_hook()
    m.get_axon_ntff_profile_hook = lambda: m._hook
    m.set_axon_ntff_profile_hook = lambda h: setattr(m, "_hook", h)
    sys.modules["antenv.axon_hooks"] = m
    antenv.axon_hooks = m


def _use_dve_exp(stage, tci, blk, h):
    """Which exp tiles go to the VectorE Schraudolph path."""
    if stage == "C":
        return h == (blk & 1) and blk < 14
    return h == (blk & 1) and blk == 5


def _build(use_mask, debug=False):
    import concourse.bass as bass
    import concourse.mybir as mybir
    import concourse.tile as tile
    from concourse import bacc
    from concourse.bass import ts

    dt = mybir.dt
    AF = mybir.ActivationFunctionType
    ALU = mybir.AluOpType

    nc = bacc.Bacc("TRN2", target_bir_lowering=False, debug=False, num_devices=NC)
    xT_d = nc.dram_tensor("xT", [C, T], dt.bfloat16, kind="ExternalInput").ap()
    yT_d = nc.dram_tensor("yT", [C, MM], dt.bfloat16, kind="ExternalInput").ap()
    # wT cols: q (pre-scaled by 1/8) 0:128, k 128:256, v 256:384
    wT_d = nc.dram_tensor("wT", [C, 384], dt.bfloat16, kind="ExternalInput").ap()
    # biases rows: 0 = bq/8, 1 = bk
    bias_d = nc.dram_tensor("biases", [2, 128, 1], dt.float32, kind="ExternalInput").ap()
    pw_d = nc.dram_tensor("projT", [128, C], dt.bfloat16, kind="ExternalInput").ap()
    if use_mask:
        mk_d = nc.dram_tensor("mask01T", [T, T], dt.bfloat16, kind="ExternalInput").ap()
    out_d = nc.dram_tensor("out_partial", [T, C], dt.float32, kind="ExternalOutput").ap()
    dbg = {}
    if debug:
        for nm, shp, ddt in [
            ("d_qx", [128, T], dt.bfloat16), ("d_ky", [128, MM], dt.bfloat16),
            ("d_kx", [128, T], dt.bfloat16),
            ("d_vy_aug", [128, 130 * NMB], dt.bfloat16),
            ("d_vx_aug", [128, 130 * NSB], dt.bfloat16),
            ("d_qy_tok", [128, MM], dt.bfloat16), ("d_ky_tok", [128, MM], dt.bfloat16),
            ("d_gt", [128, 128], dt.bfloat16), ("d_wxT", [128, T], dt.bfloat16),
            ("d_cv1n0", [64, T], dt.float32), ("d_cv1n1", [64, T], dt.float32),
            ("d_diffT", [128, T], dt.bfloat16),
        ]:
            dbg[nm] = nc.dram_tensor(nm, shp, ddt, kind="ExternalOutput").ap()

    def ts_h(h):
        return slice(64 * h, 64 * h + 64)

    with tile.TileContext(nc) as tc:
        pconst_cm = tc.tile_pool(name="pconst", bufs=1)
        pconst = pconst_cm.__enter__()
        pbig_cm = tc.tile_pool(name="pbig", bufs=1)
        pbig = pbig_cm.__enter__()
        pE_cm = tc.tile_pool(name="pE", bufs=8)
        pE = pE_cm.__enter__()
        pnorm_cm = tc.tile_pool(name="pnorm", bufs=2)
        pnorm = pnorm_cm.__enter__()
        pout_cm = tc.tile_pool(name="pout", bufs=3)
        pout = pout_cm.__enter__()
        if use_mask:
            pmk_cm = tc.tile_pool(name="pmk", bufs=2)
            pmk = pmk_cm.__enter__()

        from concourse.masks import make_identity

        ident = pconst.tile([128, 128], dt.bfloat16, tag="ident")
        make_identity(nc, ident[:])
        bq8 = pconst.tile([128, 1], dt.float32, tag="bq8")
        bk = pconst.tile([128, 1], dt.float32, tag="bk")
        projT_s = pconst.tile([128, C], dt.bfloat16, tag="projT")

        # ---- input DMAs: y-side + weights first, then x ----
        nc.sync.dma_start(bq8[:], bias_d[0])
        nc.sync.dma_start(bk[:], bias_d[1])
        nc.sync.dma_start(projT_s[:], pw_d[:])
        wT = [pconst.tile([128, 384], dt.bfloat16, tag=f"wT{i}", name=f"wT{i}") for i in range(4)]
        yT = [pconst.tile([128, MM], dt.bfloat16, tag=f"yT{i}", name=f"yT{i}") for i in range(4)]
        xT = [pconst.tile([128, T], dt.bfloat16, tag=f"xT{i}", name=f"xT{i}") for i in range(4)]
        for i in range(4):
            nc.sync.dma_start(wT[i][:], wT_d[ts(i, 128), :])
        for i in range(4):
            nc.sync.dma_start(yT[i][:], yT_d[ts(i, 128), :])
        for i in range(4):
            nc.sync.dma_start(xT[i][:], xT_d[ts(i, 128), :])

        # persistent sbuf tensors
        ky = pbig.tile([128, MM], dt.bfloat16, tag="ky")
        qx = pbig.tile([128, T], dt.bfloat16, tag="qx")
        qy = pbig.tile([128, MM], dt.bfloat16, tag="qy")
        qy_tok = pbig.tile([128, MM], dt.bfloat16, tag="qy_tok")
        ky_tok = pbig.tile([128, MM], dt.bfloat16, tag="ky_tok")
        kx = pbig.tile([128, T], dt.bfloat16, tag="kx")
        vyT = pbig.tile([128, MM], dt.bfloat16, tag="vyT")
        vxT = pbig.tile([128, T], dt.bfloat16, tag="vxT")
        vy_aug = pbig.tile([128, 130 * NMB], dt.bfloat16, tag="vy_aug")
        vx_aug = pbig.tile([128, 130 * NSB], dt.bfloat16, tag="vx_aug")
        gt = pbig.tile([128, 128], dt.bfloat16, tag="gt")
        wxT = pbig.tile([128, T], dt.bfloat16, tag="wxT")
        cv1n = [pbig.tile([64, T], dt.float32, tag=f"cv1n{h}", name=f"cv1n{h}") for h in range(2)]
        diffT = pbig.tile([128, T], dt.bfloat16, tag="diffT")

        # ones columns of the aug tiles (data-independent -> fill early)
        nc.vector.memset(vy_aug[:, 64::65], 1.0)
        nc.vector.memset(vx_aug[:, 64::65], 1.0)

        # ---- stage A: y-side projections, tok transposes, G, vy_aug, qx ----
        psA_cm = tc.tile_pool(name="psA", bufs=2, space="PSUM")
        psA = psA_cm.__enter__()
        psT_cm = tc.tile_pool(name="psT", bufs=2, space="PSUM")
        psT = psT_cm.__enter__()

        def mm_ksplit_pair(outs, lhsT, rhss, start, stop, lhsT2=None):
            # contract K=128 as two K=64 halves on alternating PE row groups.
            # The two concurrent matmuls always target different psum banks
            # (outs[0]/outs[1]); same-bank writers are separated by a slot so
            # accumulation into one bank is never concurrent.
            l2 = lhsT if lhsT2 is None else lhsT2
            nc.tensor.matmul(outs[0], lhsT, rhss[0], start=start, stop=stop)
            nc.tensor.matmul(outs[1], l2, rhss[1], start=start, stop=stop)

        def emit_proj(pool, dst, srcs, wcol, tci, bias, scale, drain):
            # dst[:, tci*1024:+1024] = (qkv proj psum + bias) * scale
            pa = pool.tile([128, 1024], dt.float32, tag=pool_tag(pool))
            for c in range(4):
                mm_ksplit_pair(
                    [pa[:, ts(0, 512)], pa[:, ts(1, 512)]],
                    wT[c][:, wcol : wcol + 128],
                    [srcs[c][:, tci * 1024 : tci * 1024 + 512],
                     srcs[c][:, tci * 1024 + 512 : tci * 1024 + 1024]],
                    start=(c == 0),
                    stop=(c == 3),
                )
            dslice = dst[:, tci * 1024 : (tci + 1) * 1024]
            if drain == "act":
                # out = in*scale + bias'  (bias' pre-multiplied by scale)
                if bias is None:
                    nc.scalar.activation(dslice, pa[:], AF.Identity, scale=scale)
                else:
                    assert scale == 1.0
                    nc.scalar.activation(dslice, pa[:], AF.Identity, bias=bias[:])
            else:
                if bias is None:
                    if scale == 1.0:
                        nc.vector.tensor_copy(dslice, pa[:])
                    else:
                        nc.vector.tensor_scalar_mul(dslice, pa[:], scale)
                else:
                    nc.vector.tensor_scalar(dslice, pa[:], bias[:], scale, ALU.add, ALU.mult)

        def pool_tag(pool):
            return "pa" if pool is psA else "pu"

        def emit_tok_tr(srct, dstt, drain):
            # token-major transpose [128, 1024]: 8 PE transposes + 1 drain
            tp = psT.tile([128, 1024], dt.bfloat16, tag="tp")
            for k in range(8):
                nc.tensor.transpose(tp[:, ts(k, 128)], srct[:, ts(k, 128)], ident[:])
            if drain == "act":
                nc.scalar.activation(dstt[:], tp[:], AF.Identity)
            else:
                nc.vector.tensor_copy(dstt[:], tp[:])

        def emit_vaug_tr(srct, aug, chunk, nblk_total):
            # transpose 8 [128,128] blocks and scatter v-dims of both heads
            # into the interleaved aug layout with one strided copy
            if srct is vyT:
                tp = psT.tile([128, 1024], dt.bfloat16, tag="tp", name="tp_vy")
                tpb = tp[:]
            else:
                # during B there are no free psum banks: view half of a
                # fp32 psU rotation tile as bf16 for the transpose target
                tp = psU.tile([128, 1024], dt.float32, tag="pu", name="tp_vx")
                tpb = tp[:].bitcast(dt.bfloat16)[:, 0:1024]
            for k in range(8):
                nc.tensor.transpose(
                    tpb[:, ts(k, 128)], srct[:, ts(chunk * 8 + k, 128)], ident[:]
                )
            src4 = tpb.rearrange("p (b h d) -> p b h d", h=2, d=64)
            dst4 = aug[:, chunk * 8 * 130 : (chunk + 1) * 8 * 130].rearrange(
                "p (b h d) -> p b h d", h=2, d=65
            )[:, :, :, 0:64]
            nc.vector.tensor_copy(dst4, src4)

        with nc.named_scope("stageA"):
            emit_proj(psA, ky, yT, 128, 0, bk, 1.0, "dve")
            emit_proj(psA, vyT, yT, 256, 0, None, 1.0, "dve")
            emit_proj(psA, qy, yT, 0, 0, bq8, 0.125, "dve")
            emit_vaug_tr(vyT, vy_aug, 0, NMB)
            emit_tok_tr(qy, qy_tok, "act")
            emit_tok_tr(ky, ky_tok, "dve")
            # G^T (block-diagonal per head): gt = sum_m qy_tok^T ky_tok
            pg = psA.tile([128, 1024], dt.float32, tag="pa")
            for mb in range(NMB):
                nc.tensor.matmul(
                    pg[:, 0:128],
                    qy_tok[:, ts(mb, 128)],
                    ky_tok[:, ts(mb, 128)],
                    start=(mb == 0),
                    stop=(mb == NMB - 1),
                )
            nc.vector.memset(gt[:], 0.0)
            nc.vector.tensor_copy(gt[0:64, 0:64], pg[0:64, 0:64])
            nc.scalar.activation(gt[64:128, 64:128], pg[64:128, 64:128], AF.Identity)
            emit_proj(psA, qx, xT, 0, 0, bq8, 1.0, "act")
            emit_proj(psA, qx, xT, 0, 1, bq8, 1.0, "act")
        psT_cm.__exit__(None, None, None)
        psA_cm.__exit__(None, None, None)

        # ---- B/C psum pools: psU 2x[128,1024] + cv0/cv1 [65,1024] ----
        psU_cm = tc.tile_pool(name="psU", bufs=2, space="PSUM")
        psU = psU_cm.__enter__()
        psCV_cm = tc.tile_pool(name="psCV", bufs=1, space="PSUM")
        psCV = psCV_cm.__enter__()

        # ---- deferred x-side work, injected into B/early-C blocks ----
        def emit_proj2(dst, wcol, tci, drain):
            emit_proj(psU, dst, xT, wcol, tci, None, 1.0, drain)

        def emit_wx(tci):
            pa = psU.tile([128, 1024], dt.float32, tag="pu")
            mm_ksplit_pair(
                [pa[:, ts(0, 512)], pa[:, ts(1, 512)]],
                gt[:],
                [kx[:, tci * 1024 : tci * 1024 + 512],
                 kx[:, tci * 1024 + 512 : tci * 1024 + 1024]],
                start=True,
                stop=True,
            )
            nc.vector.tensor_copy(wxT[:, tci * 1024 : (tci + 1) * 1024], pa[:])

        a_work = {
            ("B", 0, 1): lambda: emit_proj2(kx, 128, 0, "dve"),
            ("B", 0, 3): lambda: emit_proj2(kx, 128, 1, "act"),
            ("B", 0, 5): lambda: emit_proj2(vxT, 256, 0, "dve"),
            ("B", 0, 7): lambda: emit_wx(0),
            ("B", 1, 1): lambda: emit_vaug_tr(vxT, vx_aug, 0, NSB),
            ("B", 1, 3): lambda: emit_proj2(vxT, 256, 1, "act"),
            ("B", 1, 5): lambda: emit_vaug_tr(vxT, vx_aug, 1, NSB),
            ("B", 1, 7): lambda: emit_wx(1),
        }

        # ---- normalization ----
        deferred_norms = []

        def emit_norm_head(stage, tci, h, cv):
            # dn on ACT (idle at tci boundaries) so the reciprocal chain
            # starts immediately without queueing behind DVE exps
            dn = pnorm.tile([1, 1024], dt.float32, tag="dn")
            nc.scalar.activation(dn[:], cv[64:65, :], AF.Identity)
            r = pnorm.tile([1, 1024], dt.float32, tag="r")
            nc.vector.reciprocal_approx_fast(r[:], dn[:])
            rbs = pnorm.tile([64, 1024], dt.float32, tag="rbs")
            nc.gpsimd.partition_broadcast(rbs[:], r[:])
            return rbs

        def emit_norm_tail(stage, tci, h, cv, rbs):
            csl = slice(tci * 1024, (tci + 1) * 1024)
            if stage == "B":
                nc.vector.tensor_tensor(cv1n[h][:, csl], cv[0:64, :], rbs[:], ALU.mult)
            else:
                cv2n = pnorm.tile([64, 1024], dt.float32, tag="cv2n")
                nc.vector.tensor_tensor(cv2n[:], cv[0:64, :], rbs[:], ALU.mult)
                nc.vector.tensor_tensor(
                    diffT[ts_h(h), csl], cv1n[h][:, csl], cv2n[:], ALU.subtract
                )

        def emit_norm(stage, tci, h, cv, tail=False):
            rbs = emit_norm_head(stage, tci, h, cv)
            if tail:
                emit_norm_tail(stage, tci, h, cv, rbs)
            else:
                # defer the mult/sub so the next tci's DVE exps are not
                # queued behind them; flushed at the next tci's blk==2
                deferred_norms.append((stage, tci, h, cv, rbs))

        # ---- output projection (2 token-blocks per psum tile) ----
        def emit_proj_out(tci, p):
            pdt = psU.tile([128, 1024], dt.float32, tag="pu")
            tb0, tb1 = 8 * tci + 2 * p, 8 * tci + 2 * p + 1
            mm_ksplit_pair(
                [pdt[:, ts(0, 512)], pdt[:, ts(1, 512)]],
                diffT[:, ts(tb0, 128)],
                [projT_s[:], projT_s[:]],
                start=True, stop=True,
                lhsT2=diffT[:, ts(tb1, 128)],
            )
            o = pout.tile([128, 1024], dt.float32, tag="po")
            if p % 2 == 0:
                nc.vector.tensor_copy(o[:], pdt[:])
            else:
                nc.scalar.activation(o[:], pdt[:], AF.Identity)
            for i in range(2):
                tb = 8 * tci + 2 * p + i
                q = nc.scalar if (tci == 1 and i == 1) else nc.sync
                q.dma_start(out_d[ts(tb, 128), :], o[:, ts(i, 512)])

        proj_work = {
            ("C", 1, 3): lambda: emit_proj_out(0, 0),
            ("C", 1, 6): lambda: emit_proj_out(0, 1),
            ("C", 1, 9): lambda: emit_proj_out(0, 2),
            ("C", 1, 12): lambda: emit_proj_out(0, 3),
        }

        # ---- attention stage (shared B/C) ----
        def exp_emit(stage, tci, blk, h, e, u):
            if _use_dve_exp(stage, tci, blk, h):
                nc.vector.tensor_scalar(
                    e[:].bitcast(dt.int16), u[:], _SCH_B / _SCH_A, _SCH_A,
                    ALU.add, ALU.mult,
                )
            else:
                nc.scalar.activation(e[:], u[:], AF.Exp)

        def attn_stage(stage, lhs_sb, nblk, aug, cv_tiles):
            for tci in range(2):
                with nc.named_scope(f"{stage}{tci}"):
                    pending = []
                    for blk in range(nblk):
                        us, es = [], []
                        for h in range(2):
                            u = psU.tile([128, 1024], dt.float32, tag="pu")
                            us.append(u)
                        for j in range(2):
                            for h in range(2):
                                hh = ts_h(h)
                                nc.tensor.matmul(
                                    us[h][:, ts(j, 512)],
                                    lhs_sb[hh, ts(blk, 128)],
                                    qx[hh, tci * 1024 + j * 512 : tci * 1024 + (j + 1) * 512],
                                    start=True,
                                    stop=True,
                                )
                        for h in range(2):
                            e = pE.tile([128, 1024], dt.bfloat16, tag="e")
                            exp_emit(stage, tci, blk, h, e, us[h])
                            if use_mask and stage == "C":
                                mk = pmk.tile([128, 1024], dt.bfloat16, tag="mk")
                                nc.sync.dma_start(
                                    mk[:],
                                    mk_d[ts(blk, 128), tci * 1024 : (tci + 1) * 1024],
                                )
                                nc.vector.tensor_tensor(e[:], e[:], mk[:], ALU.mult)
                            es.append(e)
                        pending.append((blk, es))
                        if blk == 1:
                            # must precede the first cval matmul of this tci:
                            # the tails read the previous tci's cv psum
                            while deferred_norms:
                                emit_norm_tail(*deferred_norms.pop(0))
                        if len(pending) > 2:
                            emit_cv(pending.pop(0))
                        w = a_work.pop((stage, tci, blk), None)
                        if w is not None:
                            w()
                        w = proj_work.pop((stage, tci, blk), None)
                        if w is not None:
                            w()
                    for pend in pending:
                        emit_cv(pend)
                    for h in range(2):
                        emit_norm(stage, tci, h, cv_tiles[h],
                                  tail=(stage == "C" and tci == 1))
                    if stage == "C" and tci == 1:
                        while deferred_norms:
                            emit_norm_tail(*deferred_norms.pop(0))

        def make_emit_cv(stage, aug, nblk, cv_tiles):
            def emit_cv(pend):
                blk, es = pend
                for h in range(2):
                    mm_ksplit_pair(
                        [cv_tiles[h][:, ts(0, 512)], cv_tiles[h][:, ts(1, 512)]],
                        aug[:, 130 * blk + 65 * h : 130 * blk + 65 * h + 65],
                        [es[h][:, ts(0, 512)], es[h][:, ts(1, 512)]],
                        start=(blk == 0),
                        stop=(blk == nblk - 1),
                    )
            return emit_cv

        cvB = [psCV.tile([65, 1024], dt.float32, tag=f"cv{h}", name=f"cv{h}") for h in range(2)]
        emit_cv = make_emit_cv("B", vy_aug, NMB, cvB)
        attn_stage("B", ky, NMB, vy_aug, cvB)

        cvC = [psCV.tile([65, 1024], dt.float32, tag=f"cv{h}", name=f"cv{h}") for h in range(2)]
        emit_cv = make_emit_cv("C", vx_aug, NSB, cvC)
        attn_stage("C", wxT, NSB, vx_aug, cvC)

        # tail: tci=1 output projection
        with nc.named_scope("proj1"):
            for p in range(4):
                emit_proj_out(1, p)

        if debug:
            for nm, t in [
                ("d_qx", qx), ("d_ky", ky), ("d_kx", kx),
                ("d_vy_aug", vy_aug), ("d_vx_aug", vx_aug),
                ("d_qy_tok", qy_tok), ("d_ky_tok", ky_tok), ("d_gt", gt),
                ("d_wxT", wxT), ("d_cv1n0", cv1n[0]), ("d_cv1n1", cv1n[1]),
                ("d_diffT", diffT),
            ]:
                nc.sync.dma_start(dbg[nm][:], t[:])

        psCV_cm.__exit__(None, None, None)
        psU_cm.__exit__(None, None, None)
        if use_mask:
            pmk_cm.__exit__(None, None, None)
        pout_cm.__exit__(None, None, None)
        pnorm_cm.__exit__(None, None, None)
        pE_cm.__exit__(None, None, None)
        pbig_cm.__exit__(None, None, None)
        pconst_cm.__exit__(None, None, None)

    nc.compile()
    return nc


def _get_kernel(use_mask, debug=False):
    key = (use_mask, debug)
    if key not in _kernels:
        _kernels[key] = _build(use_mask, debug)
    return _kernels[key]


def _shard_inputs(x, y, attn_x_mask, qkv_w, qkv_b, proj_w, use_mask):
    import ml_dtypes

    bf16 = ml_dtypes.bfloat16
    in_maps = []
    mask01T = None
    if use_mask:
        mask01T = np.ascontiguousarray(
            np.asarray(attn_x_mask)[0, 0].T.astype(bf16)
        )
    xT_b = [np.ascontiguousarray(x[b].T.astype(bf16)) for b in range(B)]
    yT_b = [np.ascontiguousarray(y[b].T.astype(bf16)) for b in range(B)]
    for core in range(NC):
        b, hp = divmod(core, 4)
        h0, h1 = 2 * hp, 2 * hp + 1
        hs = np.r_[h0 * D : (h0 + 1) * D, h1 * D : (h1 + 1) * D]
        w_sel = qkv_w[np.r_[hs, C + hs, 2 * C + hs], :].copy()
        w_sel[0:128, :] *= 0.125  # fold 1/sqrt(D) into q weights
        m = {
            "xT": xT_b[b],
            "yT": yT_b[b],
            "wT": np.ascontiguousarray(w_sel.T.astype(bf16)),
            "biases": np.ascontiguousarray(
                np.stack([qkv_b[hs] * 0.125, qkv_b[C + hs]]).reshape(2, 128, 1)
            ).astype(np.float32),
            "projT": np.ascontiguousarray(proj_w.T[hs, :].astype(bf16)),
        }
        if use_mask:
            m["mask01T"] = mask01T
        in_maps.append(m)
    return in_maps


def _run(x, y, attn_x_mask, qkv_w, qkv_b, proj_w, proj_b, profile=False, debug=False):
    from concourse.bass_utils import run_bass_kernel_spmd

    x = np.asarray(x, np.float32)
    y = np.asarray(y, np.float32)
    qkv_w = np.asarray(qkv_w, np.float32)
    qkv_b = np.asarray(qkv_b, np.float32)
    proj_w = np.asarray(proj_w, np.float32)
    proj_b = np.asarray(proj_b, np.float32)
    mask = np.asarray(attn_x_mask)
    use_mask = not bool(mask.all())

    if profile:
        _install_ntff_hook()
    nc = _get_kernel(use_mask, debug)
    in_maps = _shard_inputs(x, y, mask, qkv_w, qkv_b, proj_w, use_mask)
    res = run_bass_kernel_spmd(nc, in_maps, list(range(NC)), trace=profile)

    out = np.zeros((B, T, C), np.float64)
    for core in range(NC):
        b = core // 4
        out[b] += res.results[core]["out_partial"].astype(np.float64)
    out += proj_b.astype(np.float64)
    return out.astype(np.float32), res


def kernel(x, y, attn_x_mask, qkv_w, qkv_b, proj_w, proj_b):
    out, _ = _run(x, y, attn_x_mask, qkv_w, qkv_b, proj_w, proj_b, profile=False)
    return out


def kernel_profiled(x, y, attn_x_mask, qkv_w, qkv_b, proj_w, proj_b):
    out, res = _run(x, y, attn_x_mask, qkv_w, qkv_b, proj_w, proj_b, profile=True)
    return out, res


# revision 29
# speedup vs baseline: 1.0267x; 1.0267x over previous
"""Trainium2 Bass kernel for nn_CrossAttentionPro (chained cross-attention).

Sharding: 8 cores = data-parallel over B (2) x head-parallel (4 head-pairs).
Each core computes, for one batch b and heads (2*hp, 2*hp+1), the full
chained cross-attention restricted to its heads, with the T x T chain
collapsed algebraically: chained = qx G kx^T with G = ky^T qy.

v2 pipeline design (vs v1):
- score matmuls (K=64) for the two heads issued to distinct PE row groups
  (base partitions 0/64) so they run concurrently.
- exp split across two engines: ScalarE exact exp and a VectorE
  Schraudolph exp2 bit-trick (i16 = A*x + B bitcast to bf16, ~2% rms err)
  so the ~12.6M exps/core are not serialized on one engine.
- cval matmuls software-pipelined one block behind the score matmuls so
  the PE never waits on the activation engines.
- softmax denominators via an appended ones-column in the v-aug slabs
  (both heads interleaved in one aug tile, scatter-filled by one strided
  copy per 8 transposed blocks).
- normalization (reciprocal-broadcast-mult-sub) mostly on GpSimd; final
  tail combo on DVE to shorten the critical path.
- x-side projections (kx/vx/wx) and the tci=0 output projection are
  injected into the attention block loops to fill PE slack.
Host sums the 4 head-pair partials per batch and adds proj_b.
"""

import math
import numpy as np

B, T, MM, C, H = 2, 2048, 1024, 512, 8
D = 64
NC = 8
NMB = MM // 128  # 8 m-blocks
NSB = T // 128  # 16 s-blocks
_kernels = {}

# Schraudolph bf16 exp2 constants: i16 = round((x*log2e)*128 + BIAS)
_LOG2E = 1.4426950408889634
_SCH_A = 128.0 * _LOG2E
_SCH_B = 16248.58


def _install_ntff_hook():
    """Bridge antenv.axon_hooks for NTFF profiling (missing in this image)."""
    import contextlib, ctypes, sys, types

    if "antenv.axon_hooks" in sys.modules:
        return
    try:
        import antenv
    except ImportError:
        return

    def _make_hook():
        try:
            lib = ctypes.CDLL("/opt/axon/libaxon_pjrt.so")
        except OSError:
            return None
        if not hasattr(lib, "axon_start_nrt_profile"):
            return None
        lib.axon_start_nrt_profile.argtypes = [
            ctypes.POINTER(ctypes.c_int64),
            ctypes.c_size_t,
        ]
        lib.axon_start_nrt_profile.restype = ctypes.c_int64
        lib.axon_stop_nrt_profile.argtypes = [ctypes.c_char_p]
        lib.axon_stop_nrt_profile.restype = ctypes.c_int64

        @contextlib.contextmanager
        def _hook(output_dir, device_ids):
            import jax

            jax.devices()
            if device_ids:
                ids = (ctypes.c_int64 * len(device_ids))(*device_ids)
                rc = lib.axon_start_nrt_profile(ids, len(device_ids))
            else:
                rc = lib.axon_start_nrt_profile(None, 0)
            if rc != 0:
                raise RuntimeError(f"axon_start_nrt_profile rc={rc}")
            try:
                yield
            finally:
                n = lib.axon_stop_nrt_profile(str(output_dir).encode())
                if n < 0:
                    raise RuntimeError(f"axon_stop_nrt_profile rc={n}")

        return _hook

    m = types.ModuleType("antenv.axon_hooks")
    m._hook = _make_hook()
    m.get_axon_ntff_profile_hook = lambda: m._hook
    m.set_axon_ntff_profile_hook = lambda h: setattr(m, "_hook", h)
    sys.modules["antenv.axon_hooks"] = m
    antenv.axon_hooks = m


def _use_dve_exp(stage, tci, blk, h):
    """Which exp tiles go to the VectorE Schraudolph path."""
    if stage == "C":
        return h == 1 and blk < 14
    return h == 1 and blk == 5


def _build(use_mask, debug=False):
    import concourse.bass as bass
    import concourse.mybir as mybir
    import concourse.tile as tile
    from concourse import bacc
    from concourse.bass import ts

    dt = mybir.dt
    AF = mybir.ActivationFunctionType
    ALU = mybir.AluOpType

    nc = bacc.Bacc("TRN2", target_bir_lowering=False, debug=False, num_devices=NC)
    xT_d = nc.dram_tensor("xT", [C, T], dt.bfloat16, kind="ExternalInput").ap()
    yT_d = nc.dram_tensor("yT", [C, MM], dt.bfloat16, kind="ExternalInput").ap()
    # wT cols: q (pre-scaled by 1/8) 0:128, k 128:256, v 256:384
    wT_d = nc.dram_tensor("wT", [C, 384], dt.bfloat16, kind="ExternalInput").ap()
    # biases rows: 0 = bq/8, 1 = bk
    bias_d = nc.dram_tensor("biases", [2, 128, 1], dt.float32, kind="ExternalInput").ap()
    pw_d = nc.dram_tensor("projT", [128, C], dt.bfloat16, kind="ExternalInput").ap()
    if use_mask:
        mk_d = nc.dram_tensor("mask01T", [T, T], dt.bfloat16, kind="ExternalInput").ap()
    out_d = nc.dram_tensor("out_partial", [T, C], dt.float32, kind="ExternalOutput").ap()
    dbg = {}
    if debug:
        for nm, shp, ddt in [
            ("d_qx", [128, T], dt.bfloat16), ("d_ky", [128, MM], dt.bfloat16),
            ("d_kx", [128, T], dt.bfloat16),
            ("d_vy_aug", [128, 130 * NMB], dt.bfloat16),
            ("d_vx_aug", [128, 130 * NSB], dt.bfloat16),
            ("d_qy_tok", [128, MM], dt.bfloat16), ("d_ky_tok", [128, MM], dt.bfloat16),
            ("d_gt", [128, 128], dt.bfloat16), ("d_wxT", [128, T], dt.bfloat16),
            ("d_cv1n0", [64, T], dt.float32), ("d_cv1n1", [64, T], dt.float32),
            ("d_diffT", [128, T], dt.bfloat16),
        ]:
            dbg[nm] = nc.dram_tensor(nm, shp, ddt, kind="ExternalOutput").ap()

    def ts_h(h):
        return slice(64 * h, 64 * h + 64)

    with tile.TileContext(nc) as tc:
        pconst_cm = tc.tile_pool(name="pconst", bufs=1)
        pconst = pconst_cm.__enter__()
        pbig_cm = tc.tile_pool(name="pbig", bufs=1)
        pbig = pbig_cm.__enter__()
        pE_cm = tc.tile_pool(name="pE", bufs=8)
        pE = pE_cm.__enter__()
        pnorm_cm = tc.tile_pool(name="pnorm", bufs=2)
        pnorm = pnorm_cm.__enter__()
        pout_cm = tc.tile_pool(name="pout", bufs=3)
        pout = pout_cm.__enter__()
        if use_mask:
            pmk_cm = tc.tile_pool(name="pmk", bufs=2)
            pmk = pmk_cm.__enter__()

        from concourse.masks import make_identity

        ident = pconst.tile([128, 128], dt.bfloat16, tag="ident")
        make_identity(nc, ident[:])
        bq8 = pconst.tile([128, 1], dt.float32, tag="bq8")
        bk = pconst.tile([128, 1], dt.float32, tag="bk")
        projT_s = pconst.tile([128, C], dt.bfloat16, tag="projT")

        # ---- input DMAs: y-side + weights first, then x ----
        nc.sync.dma_start(bq8[:], bias_d[0])
        nc.sync.dma_start(bk[:], bias_d[1])
        nc.sync.dma_start(projT_s[:], pw_d[:])
        wT = [pconst.tile([128, 384], dt.bfloat16, tag=f"wT{i}", name=f"wT{i}") for i in range(4)]
        yT = [pconst.tile([128, MM], dt.bfloat16, tag=f"yT{i}", name=f"yT{i}") for i in range(4)]
        xT = [pconst.tile([128, T], dt.bfloat16, tag=f"xT{i}", name=f"xT{i}") for i in range(4)]
        for i in range(4):
            nc.sync.dma_start(wT[i][:], wT_d[ts(i, 128), :])
        for i in range(4):
            nc.sync.dma_start(yT[i][:], yT_d[ts(i, 128), :])
        for i in range(4):
            nc.sync.dma_start(xT[i][:], xT_d[ts(i, 128), :])

        # persistent sbuf tensors
        ky = pbig.tile([128, MM], dt.bfloat16, tag="ky")
        qx = pbig.tile([128, T], dt.bfloat16, tag="qx")
        qy = pbig.tile([128, MM], dt.bfloat16, tag="qy")
        qy_tok = pbig.tile([128, MM], dt.bfloat16, tag="qy_tok")
        ky_tok = pbig.tile([128, MM], dt.bfloat16, tag="ky_tok")
        kx = pbig.tile([128, T], dt.bfloat16, tag="kx")
        vyT = pbig.tile([128, MM], dt.bfloat16, tag="vyT")
        vxT = pbig.tile([128, T], dt.bfloat16, tag="vxT")
        vy_aug = pbig.tile([128, 130 * NMB], dt.bfloat16, tag="vy_aug")
        vx_aug = pbig.tile([128, 130 * NSB], dt.bfloat16, tag="vx_aug")
        gt = pbig.tile([128, 128], dt.bfloat16, tag="gt")
        wxT = pbig.tile([128, T], dt.bfloat16, tag="wxT")
        cv1n = [pbig.tile([64, T], dt.float32, tag=f"cv1n{h}", name=f"cv1n{h}") for h in range(2)]
        diffT = pbig.tile([128, T], dt.bfloat16, tag="diffT")

        # ones columns of the aug tiles (data-independent -> fill early)
        nc.vector.memset(vy_aug[:, 64::65], 1.0)
        nc.vector.memset(vx_aug[:, 64::65], 1.0)

        # ---- stage A: y-side projections, tok transposes, G, vy_aug, qx ----
        psA_cm = tc.tile_pool(name="psA", bufs=2, space="PSUM")
        psA = psA_cm.__enter__()
        psT_cm = tc.tile_pool(name="psT", bufs=2, space="PSUM")
        psT = psT_cm.__enter__()

        def mm_ksplit_pair(outs, lhsT, rhss, start, stop, lhsT2=None):
            # contract K=128 as two K=64 halves on alternating PE row groups.
            # The two concurrent matmuls always target different psum banks
            # (outs[0]/outs[1]); same-bank writers are separated by a slot so
            # accumulation into one bank is never concurrent.
            l2 = lhsT if lhsT2 is None else lhsT2
            nc.tensor.matmul(outs[0], lhsT, rhss[0], start=start, stop=stop)
            nc.tensor.matmul(outs[1], l2, rhss[1], start=start, stop=stop)

        def emit_proj(pool, dst, srcs, wcol, tci, bias, scale, drain):
            # dst[:, tci*1024:+1024] = (qkv proj psum + bias) * scale
            pa = pool.tile([128, 1024], dt.float32, tag=pool_tag(pool))
            for c in range(4):
                mm_ksplit_pair(
                    [pa[:, ts(0, 512)], pa[:, ts(1, 512)]],
                    wT[c][:, wcol : wcol + 128],
                    [srcs[c][:, tci * 1024 : tci * 1024 + 512],
                     srcs[c][:, tci * 1024 + 512 : tci * 1024 + 1024]],
                    start=(c == 0),
                    stop=(c == 3),
                )
            dslice = dst[:, tci * 1024 : (tci + 1) * 1024]
            if drain == "act":
                # out = in*scale + bias'  (bias' pre-multiplied by scale)
                if bias is None:
                    nc.scalar.activation(dslice, pa[:], AF.Identity, scale=scale)
                else:
                    assert scale == 1.0
                    nc.scalar.activation(dslice, pa[:], AF.Identity, bias=bias[:])
            else:
                if bias is None:
                    if scale == 1.0:
                        nc.vector.tensor_copy(dslice, pa[:])
                    else:
                        nc.vector.tensor_scalar_mul(dslice, pa[:], scale)
                else:
                    nc.vector.tensor_scalar(dslice, pa[:], bias[:], scale, ALU.add, ALU.mult)

        def pool_tag(pool):
            return "pa" if pool is psA else "pu"

        def emit_tok_tr(srct, dstt, drain):
            # token-major transpose [128, 1024]: 8 PE transposes + 1 drain
            tp = psT.tile([128, 1024], dt.bfloat16, tag="tp")
            for k in range(8):
                nc.tensor.transpose(tp[:, ts(k, 128)], srct[:, ts(k, 128)], ident[:])
            if drain == "act":
                nc.scalar.activation(dstt[:], tp[:], AF.Identity)
            else:
                nc.vector.tensor_copy(dstt[:], tp[:])

        def emit_vaug_tr(srct, aug, chunk, nblk_total):
            # transpose 8 [128,128] blocks and scatter v-dims of both heads
            # into the interleaved aug layout with one strided copy
            if srct is vyT:
                tp = psT.tile([128, 1024], dt.bfloat16, tag="tp", name="tp_vy")
                tpb = tp[:]
            else:
                # during B there are no free psum banks: view half of a
                # fp32 psU rotation tile as bf16 for the transpose target
                tp = psU.tile([128, 1024], dt.float32, tag="pu", name="tp_vx")
                tpb = tp[:].bitcast(dt.bfloat16)[:, 0:1024]
            for k in range(8):
                nc.tensor.transpose(
                    tpb[:, ts(k, 128)], srct[:, ts(chunk * 8 + k, 128)], ident[:]
                )
            src4 = tpb.rearrange("p (b h d) -> p b h d", h=2, d=64)
            dst4 = aug[:, chunk * 8 * 130 : (chunk + 1) * 8 * 130].rearrange(
                "p (b h d) -> p b h d", h=2, d=65
            )[:, :, :, 0:64]
            nc.vector.tensor_copy(dst4, src4)

        with nc.named_scope("stageA"):
            emit_proj(psA, ky, yT, 128, 0, bk, 1.0, "dve")
            emit_proj(psA, vyT, yT, 256, 0, None, 1.0, "dve")
            emit_proj(psA, qy, yT, 0, 0, bq8, 0.125, "dve")
            emit_vaug_tr(vyT, vy_aug, 0, NMB)
            emit_tok_tr(qy, qy_tok, "act")
            emit_tok_tr(ky, ky_tok, "dve")
            # G^T (block-diagonal per head): gt = sum_m qy_tok^T ky_tok
            pg = psA.tile([128, 1024], dt.float32, tag="pa")
            for mb in range(NMB):
                nc.tensor.matmul(
                    pg[:, 0:128],
                    qy_tok[:, ts(mb, 128)],
                    ky_tok[:, ts(mb, 128)],
                    start=(mb == 0),
                    stop=(mb == NMB - 1),
                )
            nc.vector.memset(gt[:], 0.0)
            nc.vector.tensor_copy(gt[0:64, 0:64], pg[0:64, 0:64])
            nc.scalar.activation(gt[64:128, 64:128], pg[64:128, 64:128], AF.Identity)
            emit_proj(psA, qx, xT, 0, 0, bq8, 1.0, "act")
            emit_proj(psA, qx, xT, 0, 1, bq8, 1.0, "act")
        psT_cm.__exit__(None, None, None)
        psA_cm.__exit__(None, None, None)

        # ---- B/C psum pools: psU 2x[128,1024] + cv0/cv1 [65,1024] ----
        psU_cm = tc.tile_pool(name="psU", bufs=2, space="PSUM")
        psU = psU_cm.__enter__()
        psCV_cm = tc.tile_pool(name="psCV", bufs=1, space="PSUM")
        psCV = psCV_cm.__enter__()

        # ---- deferred x-side work, injected into B/early-C blocks ----
        def emit_proj2(dst, wcol, tci, drain):
            emit_proj(psU, dst, xT, wcol, tci, None, 1.0, drain)

        def emit_wx(tci):
            pa = psU.tile([128, 1024], dt.float32, tag="pu")
            mm_ksplit_pair(
                [pa[:, ts(0, 512)], pa[:, ts(1, 512)]],
                gt[:],
                [kx[:, tci * 1024 : tci * 1024 + 512],
                 kx[:, tci * 1024 + 512 : tci * 1024 + 1024]],
                start=True,
                stop=True,
            )
            nc.vector.tensor_copy(wxT[:, tci * 1024 : (tci + 1) * 1024], pa[:])

        a_work = {
            ("B", 0, 1): lambda: emit_proj2(kx, 128, 0, "dve"),
            ("B", 0, 3): lambda: emit_proj2(kx, 128, 1, "act"),
            ("B", 0, 5): lambda: emit_proj2(vxT, 256, 0, "dve"),
            ("B", 0, 7): lambda: emit_wx(0),
            ("B", 1, 1): lambda: emit_vaug_tr(vxT, vx_aug, 0, NSB),
            ("B", 1, 3): lambda: emit_proj2(vxT, 256, 1, "act"),
            ("B", 1, 5): lambda: emit_vaug_tr(vxT, vx_aug, 1, NSB),
            ("B", 1, 7): lambda: emit_wx(1),
        }

        # ---- normalization ----
        deferred_norms = []

        def emit_norm_head(stage, tci, h, cv):
            # dn on ACT (idle at tci boundaries) so the reciprocal chain
            # starts immediately without queueing behind DVE exps
            dn = pnorm.tile([1, 1024], dt.float32, tag="dn")
            nc.scalar.activation(dn[:], cv[64:65, :], AF.Identity)
            r = pnorm.tile([1, 1024], dt.float32, tag="r")
            nc.vector.reciprocal_approx_fast(r[:], dn[:])
            rbs = pnorm.tile([64, 1024], dt.float32, tag="rbs")
            nc.gpsimd.partition_broadcast(rbs[:], r[:])
            return rbs

        def emit_norm_tail(stage, tci, h, cv, rbs):
            csl = slice(tci * 1024, (tci + 1) * 1024)
            if stage == "B":
                nc.vector.tensor_tensor(cv1n[h][:, csl], cv[0:64, :], rbs[:], ALU.mult)
            else:
                cv2n = pnorm.tile([64, 1024], dt.float32, tag="cv2n")
                nc.vector.tensor_tensor(cv2n[:], cv[0:64, :], rbs[:], ALU.mult)
                nc.vector.tensor_tensor(
                    diffT[ts_h(h), csl], cv1n[h][:, csl], cv2n[:], ALU.subtract
                )

        def emit_norm(stage, tci, h, cv, tail=False):
            rbs = emit_norm_head(stage, tci, h, cv)
            if tail:
                emit_norm_tail(stage, tci, h, cv, rbs)
            else:
                # defer the mult/sub so the next tci's DVE exps are not
                # queued behind them; flushed at the next tci's blk==2
                deferred_norms.append((stage, tci, h, cv, rbs))

        # ---- output projection (2 token-blocks per psum tile) ----
        def emit_proj_out(tci, p):
            pdt = psU.tile([128, 1024], dt.float32, tag="pu")
            tb0, tb1 = 8 * tci + 2 * p, 8 * tci + 2 * p + 1
            mm_ksplit_pair(
                [pdt[:, ts(0, 512)], pdt[:, ts(1, 512)]],
                diffT[:, ts(tb0, 128)],
                [projT_s[:], projT_s[:]],
                start=True, stop=True,
                lhsT2=diffT[:, ts(tb1, 128)],
            )
            o = pout.tile([128, 1024], dt.float32, tag="po")
            if p % 2 == 0:
                nc.vector.tensor_copy(o[:], pdt[:])
            else:
                nc.scalar.activation(o[:], pdt[:], AF.Identity)
            for i in range(2):
                tb = 8 * tci + 2 * p + i
                q = nc.scalar if (tci == 1 and i == 1) else nc.sync
                q.dma_start(out_d[ts(tb, 128), :], o[:, ts(i, 512)])

        proj_work = {
            ("C", 1, 3): lambda: emit_proj_out(0, 0),
            ("C", 1, 6): lambda: emit_proj_out(0, 1),
            ("C", 1, 9): lambda: emit_proj_out(0, 2),
            ("C", 1, 12): lambda: emit_proj_out(0, 3),
        }

        # ---- attention stage (shared B/C) ----
        def exp_emit(stage, tci, blk, h, e, u):
            if _use_dve_exp(stage, tci, blk, h):
                nc.vector.tensor_scalar(
                    e[:].bitcast(dt.int16), u[:], _SCH_B / _SCH_A, _SCH_A,
                    ALU.add, ALU.mult,
                )
            else:
                nc.scalar.activation(e[:], u[:], AF.Exp)

        def attn_stage(stage, lhs_sb, nblk, aug, cv_tiles):
            for tci in range(2):
                with nc.named_scope(f"{stage}{tci}"):
                    pending = []
                    for blk in range(nblk):
                        us, es = [], []
                        for h in range(2):
                            u = psU.tile([128, 1024], dt.float32, tag="pu")
                            us.append(u)
                        for j in range(2):
                            for h in range(2):
                                hh = ts_h(h)
                                nc.tensor.matmul(
                                    us[h][:, ts(j, 512)],
                                    lhs_sb[hh, ts(blk, 128)],
                                    qx[hh, tci * 1024 + j * 512 : tci * 1024 + (j + 1) * 512],
                                    start=True,
                                    stop=True,
                                )
                        for h in range(2):
                            e = pE.tile([128, 1024], dt.bfloat16, tag="e")
                            exp_emit(stage, tci, blk, h, e, us[h])
                            if use_mask and stage == "C":
                                mk = pmk.tile([128, 1024], dt.bfloat16, tag="mk")
                                nc.sync.dma_start(
                                    mk[:],
                                    mk_d[ts(blk, 128), tci * 1024 : (tci + 1) * 1024],
                                )
                                nc.vector.tensor_tensor(e[:], e[:], mk[:], ALU.mult)
                            es.append(e)
                        pending.append((blk, es))
                        if blk == 1:
                            # must precede the first cval matmul of this tci:
                            # the tails read the previous tci's cv psum
                            while deferred_norms:
                                emit_norm_tail(*deferred_norms.pop(0))
                        if len(pending) > 2:
                            emit_cv(pending.pop(0))
                        w = a_work.pop((stage, tci, blk), None)
                        if w is not None:
                            w()
                        w = proj_work.pop((stage, tci, blk), None)
                        if w is not None:
                            w()
                    for pend in pending:
                        emit_cv(pend)
                    for h in range(2):
                        emit_norm(stage, tci, h, cv_tiles[h],
                                  tail=(stage == "C" and tci == 1))
                    if stage == "C" and tci == 1:
                        while deferred_norms:
                            emit_norm_tail(*deferred_norms.pop(0))

        def make_emit_cv(stage, aug, nblk, cv_tiles):
            def emit_cv(pend):
                blk, es = pend
                for h in range(2):
                    mm_ksplit_pair(
                        [cv_tiles[h][:, ts(0, 512)], cv_tiles[h][:, ts(1, 512)]],
                        aug[:, 130 * blk + 65 * h : 130 * blk + 65 * h + 65],
                        [es[h][:, ts(0, 512)], es[h][:, ts(1, 512)]],
                        start=(blk == 0),
                        stop=(blk == nblk - 1),
                    )
            return emit_cv

        cvB = [psCV.tile([65, 1024], dt.float32, tag=f"cv{h}", name=f"cv{h}") for h in range(2)]
        emit_cv = make_emit_cv("B", vy_aug, NMB, cvB)
        attn_stage("B", ky, NMB, vy_aug, cvB)

        cvC = [psCV.tile([65, 1024], dt.float32, tag=f"cv{h}", name=f"cv{h}") for h in range(2)]
        emit_cv = make_emit_cv("C", vx_aug, NSB, cvC)
        attn_stage("C", wxT, NSB, vx_aug, cvC)

        # tail: tci=1 output projection
        with nc.named_scope("proj1"):
            for p in range(4):
                emit_proj_out(1, p)

        if debug:
            for nm, t in [
                ("d_qx", qx), ("d_ky", ky), ("d_kx", kx),
                ("d_vy_aug", vy_aug), ("d_vx_aug", vx_aug),
                ("d_qy_tok", qy_tok), ("d_ky_tok", ky_tok), ("d_gt", gt),
                ("d_wxT", wxT), ("d_cv1n0", cv1n[0]), ("d_cv1n1", cv1n[1]),
                ("d_diffT", diffT),
            ]:
                nc.sync.dma_start(dbg[nm][:], t[:])

        psCV_cm.__exit__(None, None, None)
        psU_cm.__exit__(None, None, None)
        if use_mask:
            pmk_cm.__exit__(None, None, None)
        pout_cm.__exit__(None, None, None)
        pnorm_cm.__exit__(None, None, None)
        pE_cm.__exit__(None, None, None)
        pbig_cm.__exit__(None, None, None)
        pconst_cm.__exit__(None, None, None)

    nc.compile()
    return nc


def _get_kernel(use_mask, debug=False):
    key = (use_mask, debug)
    if key not in _kernels:
        _kernels[key] = _build(use_mask, debug)
    return _kernels[key]


def _shard_inputs(x, y, attn_x_mask, qkv_w, qkv_b, proj_w, use_mask):
    import ml_dtypes

    bf16 = ml_dtypes.bfloat16
    in_maps = []
    mask01T = None
    if use_mask:
        mask01T = np.ascontiguousarray(
            np.asarray(attn_x_mask)[0, 0].T.astype(bf16)
        )
    xT_b = [np.ascontiguousarray(x[b].T.astype(bf16)) for b in range(B)]
    yT_b = [np.ascontiguousarray(y[b].T.astype(bf16)) for b in range(B)]
    for core in range(NC):
        b, hp = divmod(core, 4)
        h0, h1 = 2 * hp, 2 * hp + 1
        hs = np.r_[h0 * D : (h0 + 1) * D, h1 * D : (h1 + 1) * D]
        w_sel = qkv_w[np.r_[hs, C + hs, 2 * C + hs], :].copy()
        w_sel[0:128, :] *= 0.125  # fold 1/sqrt(D) into q weights
        m = {
            "xT": xT_b[b],
            "yT": yT_b[b],
            "wT": np.ascontiguousarray(w_sel.T.astype(bf16)),
            "biases": np.ascontiguousarray(
                np.stack([qkv_b[hs] * 0.125, qkv_b[C + hs]]).reshape(2, 128, 1)
            ).astype(np.float32),
            "projT": np.ascontiguousarray(proj_w.T[hs, :].astype(bf16)),
        }
        if use_mask:
            m["mask01T"] = mask01T
        in_maps.append(m)
    return in_maps


def _run(x, y, attn_x_mask, qkv_w, qkv_b, proj_w, proj_b, profile=False, debug=False):
    from concourse.bass_utils import run_bass_kernel_spmd

    x = np.asarray(x, np.float32)
    y = np.asarray(y, np.float32)
    qkv_w = np.asarray(qkv_w, np.float32)
    qkv_b = np.asarray(qkv_b, np.float32)
    proj_w = np.asarray(proj_w, np.float32)
    proj_b = np.asarray(proj_b, np.float32)
    mask = np.asarray(attn_x_mask)
    use_mask = not bool(mask.all())

    if profile:
        _install_ntff_hook()
    nc = _get_kernel(use_mask, debug)
    in_maps = _shard_inputs(x, y, mask, qkv_w, qkv_b, proj_w, use_mask)
    res = run_bass_kernel_spmd(nc, in_maps, list(range(NC)), trace=profile)

    out = np.zeros((B, T, C), np.float64)
    for core in range(NC):
        b = core // 4
        out[b] += res.results[core]["out_partial"].astype(np.float64)
    out += proj_b.astype(np.float64)
    return out.astype(np.float32), res


def kernel(x, y, attn_x_mask, qkv_w, qkv_b, proj_w, proj_b):
    out, _ = _run(x, y, attn_x_mask, qkv_w, qkv_b, proj_w, proj_b, profile=False)
    return out


def kernel_profiled(x, y, attn_x_mask, qkv_w, qkv_b, proj_w, proj_b):
    out, res = _run(x, y, attn_x_mask, qkv_w, qkv_b, proj_w, proj_b, profile=True)
    return out, res


# revision 31
# speedup vs baseline: 1.0426x; 1.0155x over previous
"""Trainium2 Bass kernel for nn_CrossAttentionPro (chained cross-attention).

Sharding: 8 cores = data-parallel over B (2) x head-parallel (4 head-pairs).
Each core computes, for one batch b and heads (2*hp, 2*hp+1), the full
chained cross-attention restricted to its heads, with the T x T chain
collapsed algebraically: chained = qx G kx^T with G = ky^T qy.

v2 pipeline design (vs v1):
- score matmuls (K=64) for the two heads issued to distinct PE row groups
  (base partitions 0/64) so they run concurrently.
- exp split across two engines: ScalarE exact exp and a VectorE
  Schraudolph exp2 bit-trick (i16 = A*x + B bitcast to bf16, ~2% rms err)
  so the ~12.6M exps/core are not serialized on one engine.
- cval matmuls software-pipelined two blocks behind the score matmuls so
  the PE never waits on the activation engines.
- softmax denominators via an appended ones-column in the v-aug slabs
  (both heads interleaved in one aug tile, scatter-filled by one strided
  copy per 8 transposed blocks).
- normalization split into a head (dn copy on ScalarE, reciprocal on DVE,
  partition-broadcast as GpSimd's only op type to avoid ext-isa library
  reloads) and a deferred tail (mult/sub on DVE) that is emitted after the
  next tci's first pairs so exps are never queued behind it.
- x-side projections (kx/vx/wx) and the tci=0 output projection are
  injected into the attention block loops to fill PE slack.
Host sums the 4 head-pair partials per batch and adds proj_b.
"""

import math
import numpy as np

B, T, MM, C, H = 2, 2048, 1024, 512, 8
D = 64
NC = 8
NMB = MM // 128  # 8 m-blocks
NSB = T // 128  # 16 s-blocks
_kernels = {}

# Schraudolph bf16 exp2 constants: i16 = round((x*log2e)*128 + BIAS)
_LOG2E = 1.4426950408889634
_SCH_A = 128.0 * _LOG2E
_SCH_B = 16248.58


def _install_ntff_hook():
    """Bridge antenv.axon_hooks for NTFF profiling (missing in this image)."""
    import contextlib, ctypes, sys, types

    if "antenv.axon_hooks" in sys.modules:
        return
    try:
        import antenv
    except ImportError:
        return

    def _make_hook():
        try:
            lib = ctypes.CDLL("/opt/axon/libaxon_pjrt.so")
        except OSError:
            return None
        if not hasattr(lib, "axon_start_nrt_profile"):
            return None
        lib.axon_start_nrt_profile.argtypes = [
            ctypes.POINTER(ctypes.c_int64),
            ctypes.c_size_t,
        ]
        lib.axon_start_nrt_profile.restype = ctypes.c_int64
        lib.axon_stop_nrt_profile.argtypes = [ctypes.c_char_p]
        lib.axon_stop_nrt_profile.restype = ctypes.c_int64

        @contextlib.contextmanager
        def _hook(output_dir, device_ids):
            import jax

            jax.devices()
            if device_ids:
                ids = (ctypes.c_int64 * len(device_ids))(*device_ids)
                rc = lib.axon_start_nrt_profile(ids, len(device_ids))
            else:
                rc = lib.axon_start_nrt_profile(None, 0)
            if rc != 0:
                raise RuntimeError(f"axon_start_nrt_profile rc={rc}")
            try:
                yield
            finally:
                n = lib.axon_stop_nrt_profile(str(output_dir).encode())
                if n < 0:
                    raise RuntimeError(f"axon_stop_nrt_profile rc={n}")

        return _hook

    m = types.ModuleType("antenv.axon_hooks")
    m._hook = _make_hook()
    m.get_axon_ntff_profile_hook = lambda: m._hook
    m.set_axon_ntff_profile_hook = lambda h: setattr(m, "_hook", h)
    sys.modules["antenv.axon_hooks"] = m
    antenv.axon_hooks = m


def _use_dve_exp(stage, tci, blk, h):
    """Which exp tiles go to the VectorE Schraudolph path."""
    if stage == "C":
        return h == 1 and blk < 13
    return h == 1 and blk % 2 == 1


def _build(use_mask, debug=False):
    import concourse.bass as bass
    import concourse.mybir as mybir
    import concourse.tile as tile
    from concourse import bacc
    from concourse.bass import ts

    dt = mybir.dt
    AF = mybir.ActivationFunctionType
    ALU = mybir.AluOpType

    nc = bacc.Bacc("TRN2", target_bir_lowering=False, debug=False, num_devices=NC)
    xT_d = nc.dram_tensor("xT", [C, T], dt.bfloat16, kind="ExternalInput").ap()
    yT_d = nc.dram_tensor("yT", [C, MM], dt.bfloat16, kind="ExternalInput").ap()
    # wT cols: q (pre-scaled by 1/8) 0:128, k 128:256, v 256:384
    wT_d = nc.dram_tensor("wT", [C, 384], dt.bfloat16, kind="ExternalInput").ap()
    # biases rows: 0 = bq/8, 1 = bk
    bias_d = nc.dram_tensor("biases", [2, 128, 1], dt.float32, kind="ExternalInput").ap()
    pw_d = nc.dram_tensor("projT", [128, C], dt.bfloat16, kind="ExternalInput").ap()
    if use_mask:
        mk_d = nc.dram_tensor("mask01T", [T, T], dt.bfloat16, kind="ExternalInput").ap()
    out_d = nc.dram_tensor("out_partial", [T, C], dt.float32, kind="ExternalOutput").ap()
    dbg = {}
    if debug:
        for nm, shp, ddt in [
            ("d_qx", [128, T], dt.bfloat16), ("d_ky", [128, MM], dt.bfloat16),
            ("d_kx", [128, T], dt.bfloat16),
            ("d_vy_aug", [128, 130 * NMB], dt.bfloat16),
            ("d_vx_aug", [128, 130 * NSB], dt.bfloat16),
            ("d_qy_tok", [128, MM], dt.bfloat16), ("d_ky_tok", [128, MM], dt.bfloat16),
            ("d_gt", [128, 128], dt.bfloat16), ("d_wxT", [128, T], dt.bfloat16),
            ("d_cv1n0", [64, T], dt.float32), ("d_cv1n1", [64, T], dt.float32),
            ("d_diffT", [128, T], dt.bfloat16),
        ]:
            dbg[nm] = nc.dram_tensor(nm, shp, ddt, kind="ExternalOutput").ap()

    def ts_h(h):
        return slice(64 * h, 64 * h + 64)

    with tile.TileContext(nc) as tc:
        pconst_cm = tc.tile_pool(name="pconst", bufs=1)
        pconst = pconst_cm.__enter__()
        pbig_cm = tc.tile_pool(name="pbig", bufs=1)
        pbig = pbig_cm.__enter__()
        pE_cm = tc.tile_pool(name="pE", bufs=8)
        pE = pE_cm.__enter__()
        pnorm_cm = tc.tile_pool(name="pnorm", bufs=2)
        pnorm = pnorm_cm.__enter__()
        pout_cm = tc.tile_pool(name="pout", bufs=3)
        pout = pout_cm.__enter__()
        if use_mask:
            pmk_cm = tc.tile_pool(name="pmk", bufs=2)
            pmk = pmk_cm.__enter__()

        from concourse.masks import make_identity

        ident = pconst.tile([128, 128], dt.bfloat16, tag="ident")
        make_identity(nc, ident[:])
        bq8 = pconst.tile([128, 1], dt.float32, tag="bq8")
        bk = pconst.tile([128, 1], dt.float32, tag="bk")
        projT_s = pconst.tile([128, C], dt.bfloat16, tag="projT")

        # ---- input DMAs: y-side + weights first, then x ----
        nc.sync.dma_start(bq8[:], bias_d[0])
        nc.sync.dma_start(bk[:], bias_d[1])
        nc.sync.dma_start(projT_s[:], pw_d[:])
        wT = [pconst.tile([128, 384], dt.bfloat16, tag=f"wT{i}", name=f"wT{i}") for i in range(4)]
        yT = [pconst.tile([128, MM], dt.bfloat16, tag=f"yT{i}", name=f"yT{i}") for i in range(4)]
        xT = [pconst.tile([128, T], dt.bfloat16, tag=f"xT{i}", name=f"xT{i}") for i in range(4)]
        for i in range(4):
            nc.sync.dma_start(wT[i][:], wT_d[ts(i, 128), :])
        for i in range(4):
            nc.sync.dma_start(yT[i][:], yT_d[ts(i, 128), :])
        for i in range(4):
            nc.sync.dma_start(xT[i][:], xT_d[ts(i, 128), :])

        # persistent sbuf tensors
        ky = pbig.tile([128, MM], dt.bfloat16, tag="ky")
        qx = pbig.tile([128, T], dt.bfloat16, tag="qx")
        qy = pbig.tile([128, MM], dt.bfloat16, tag="qy")
        qy_tok = pbig.tile([128, MM], dt.bfloat16, tag="qy_tok")
        ky_tok = pbig.tile([128, MM], dt.bfloat16, tag="ky_tok")
        kx = pbig.tile([128, T], dt.bfloat16, tag="kx")
        vyT = pbig.tile([128, MM], dt.bfloat16, tag="vyT")
        vxT = pbig.tile([128, T], dt.bfloat16, tag="vxT")
        vy_aug = pbig.tile([128, 130 * NMB], dt.bfloat16, tag="vy_aug")
        vx_aug = pbig.tile([128, 130 * NSB], dt.bfloat16, tag="vx_aug")
        gt = pbig.tile([128, 128], dt.bfloat16, tag="gt")
        wxT = pbig.tile([128, T], dt.bfloat16, tag="wxT")
        cv1n = [pbig.tile([64, T], dt.float32, tag=f"cv1n{h}", name=f"cv1n{h}") for h in range(2)]
        diffT = pbig.tile([128, T], dt.bfloat16, tag="diffT")

        # ones columns of the aug tiles (data-independent -> fill early)
        nc.vector.memset(vy_aug[:, 64::65], 1.0)
        nc.vector.memset(vx_aug[:, 64::65], 1.0)

        # ---- stage A: y-side projections, tok transposes, G, vy_aug, qx ----
        psA_cm = tc.tile_pool(name="psA", bufs=2, space="PSUM")
        psA = psA_cm.__enter__()
        psT_cm = tc.tile_pool(name="psT", bufs=2, space="PSUM")
        psT = psT_cm.__enter__()

        def mm_ksplit_pair(outs, lhsT, rhss, start, stop, lhsT2=None):
            # contract K=128 as two K=64 halves on alternating PE row groups.
            # The two concurrent matmuls always target different psum banks
            # (outs[0]/outs[1]); same-bank writers are separated by a slot so
            # accumulation into one bank is never concurrent.
            l2 = lhsT if lhsT2 is None else lhsT2
            nc.tensor.matmul(outs[0], lhsT, rhss[0], start=start, stop=stop)
            nc.tensor.matmul(outs[1], l2, rhss[1], start=start, stop=stop)

        def emit_proj(pool, dst, srcs, wcol, tci, bias, scale, drain):
            # dst[:, tci*1024:+1024] = (qkv proj psum + bias) * scale
            pa = pool.tile([128, 1024], dt.float32, tag=pool_tag(pool))
            for c in range(4):
                mm_ksplit_pair(
                    [pa[:, ts(0, 512)], pa[:, ts(1, 512)]],
                    wT[c][:, wcol : wcol + 128],
                    [srcs[c][:, tci * 1024 : tci * 1024 + 512],
                     srcs[c][:, tci * 1024 + 512 : tci * 1024 + 1024]],
                    start=(c == 0),
                    stop=(c == 3),
                )
            dslice = dst[:, tci * 1024 : (tci + 1) * 1024]
            if drain == "act":
                # out = in*scale + bias'  (bias' pre-multiplied by scale)
                if bias is None:
                    nc.scalar.activation(dslice, pa[:], AF.Identity, scale=scale)
                else:
                    assert scale == 1.0
                    nc.scalar.activation(dslice, pa[:], AF.Identity, bias=bias[:])
            else:
                if bias is None:
                    if scale == 1.0:
                        nc.vector.tensor_copy(dslice, pa[:])
                    else:
                        nc.vector.tensor_scalar_mul(dslice, pa[:], scale)
                else:
                    nc.vector.tensor_scalar(dslice, pa[:], bias[:], scale, ALU.add, ALU.mult)

        def pool_tag(pool):
            return "pa" if pool is psA else "pu"

        def emit_tok_tr(srct, dstt, drain):
            # token-major transpose [128, 1024]: 8 PE transposes + 1 drain
            tp = psT.tile([128, 1024], dt.bfloat16, tag="tp")
            for k in range(8):
                nc.tensor.transpose(tp[:, ts(k, 128)], srct[:, ts(k, 128)], ident[:])
            if drain == "act":
                nc.scalar.activation(dstt[:], tp[:], AF.Identity)
            else:
                nc.vector.tensor_copy(dstt[:], tp[:])

        def emit_vaug_tr(srct, aug, chunk, nblk_total):
            # transpose 8 [128,128] blocks and scatter v-dims of both heads
            # into the interleaved aug layout with one strided copy
            if srct is vyT:
                tp = psT.tile([128, 1024], dt.bfloat16, tag="tp", name="tp_vy")
                tpb = tp[:]
            else:
                # during B there are no free psum banks: view half of a
                # fp32 psU rotation tile as bf16 for the transpose target
                tp = psU.tile([128, 1024], dt.float32, tag="pu", name="tp_vx")
                tpb = tp[:].bitcast(dt.bfloat16)[:, 0:1024]
            for k in range(8):
                nc.tensor.transpose(
                    tpb[:, ts(k, 128)], srct[:, ts(chunk * 8 + k, 128)], ident[:]
                )
            src4 = tpb.rearrange("p (b h d) -> p b h d", h=2, d=64)
            dst4 = aug[:, chunk * 8 * 130 : (chunk + 1) * 8 * 130].rearrange(
                "p (b h d) -> p b h d", h=2, d=65
            )[:, :, :, 0:64]
            nc.vector.tensor_copy(dst4, src4)

        with nc.named_scope("stageA"):
            emit_proj(psA, ky, yT, 128, 0, bk, 1.0, "dve")
            emit_proj(psA, vyT, yT, 256, 0, None, 1.0, "dve")
            emit_proj(psA, qy, yT, 0, 0, bq8, 0.125, "dve")
            emit_vaug_tr(vyT, vy_aug, 0, NMB)
            emit_tok_tr(qy, qy_tok, "act")
            emit_tok_tr(ky, ky_tok, "dve")
            # G^T (block-diagonal per head): gt = sum_m qy_tok^T ky_tok
            pg = psA.tile([128, 1024], dt.float32, tag="pa")
            for mb in range(NMB):
                nc.tensor.matmul(
                    pg[:, 0:128],
                    qy_tok[:, ts(mb, 128)],
                    ky_tok[:, ts(mb, 128)],
                    start=(mb == 0),
                    stop=(mb == NMB - 1),
                )
            nc.vector.memset(gt[:], 0.0)
            nc.vector.tensor_copy(gt[0:64, 0:64], pg[0:64, 0:64])
            nc.scalar.activation(gt[64:128, 64:128], pg[64:128, 64:128], AF.Identity)
            emit_proj(psA, qx, xT, 0, 0, bq8, 1.0, "act")
            emit_proj(psA, qx, xT, 0, 1, bq8, 1.0, "act")
        psT_cm.__exit__(None, None, None)
        psA_cm.__exit__(None, None, None)

        # ---- B/C psum pools: psU 2x[128,1024] + cv0/cv1 [65,1024] ----
        psU_cm = tc.tile_pool(name="psU", bufs=2, space="PSUM")
        psU = psU_cm.__enter__()
        psCV_cm = tc.tile_pool(name="psCV", bufs=1, space="PSUM")
        psCV = psCV_cm.__enter__()

        # ---- deferred x-side work, injected into B/early-C blocks ----
        def emit_proj2(dst, wcol, tci, drain):
            emit_proj(psU, dst, xT, wcol, tci, None, 1.0, drain)

        def emit_wx(tci):
            pa = psU.tile([128, 1024], dt.float32, tag="pu")
            mm_ksplit_pair(
                [pa[:, ts(0, 512)], pa[:, ts(1, 512)]],
                gt[:],
                [kx[:, tci * 1024 : tci * 1024 + 512],
                 kx[:, tci * 1024 + 512 : tci * 1024 + 1024]],
                start=True,
                stop=True,
            )
            nc.vector.tensor_copy(wxT[:, tci * 1024 : (tci + 1) * 1024], pa[:])

        a_work = {
            ("B", 0, 1): lambda: emit_proj2(kx, 128, 0, "dve"),
            ("B", 0, 3): lambda: emit_proj2(kx, 128, 1, "act"),
            ("B", 0, 5): lambda: emit_proj2(vxT, 256, 0, "dve"),
            ("B", 0, 7): lambda: emit_wx(0),
            ("B", 1, 1): lambda: emit_vaug_tr(vxT, vx_aug, 0, NSB),
            ("B", 1, 3): lambda: emit_proj2(vxT, 256, 1, "act"),
            ("B", 1, 5): lambda: emit_vaug_tr(vxT, vx_aug, 1, NSB),
            ("B", 1, 7): lambda: emit_wx(1),
        }

        # ---- normalization ----
        deferred_norms = []

        def emit_norm_head(stage, tci, h, cv):
            # dn on ACT (idle at tci boundaries) so the reciprocal chain
            # starts immediately without queueing behind DVE exps
            dn = pnorm.tile([1, 1024], dt.float32, tag="dn")
            nc.scalar.activation(dn[:], cv[64:65, :], AF.Identity)
            r = pnorm.tile([1, 1024], dt.float32, tag="r")
            nc.vector.reciprocal_approx_fast(r[:], dn[:])
            rbs = pnorm.tile([64, 1024], dt.float32, tag="rbs")
            nc.gpsimd.partition_broadcast(rbs[:], r[:])
            return rbs

        def emit_norm_tail(stage, tci, h, cv, rbs):
            csl = slice(tci * 1024, (tci + 1) * 1024)
            if stage == "B":
                nc.vector.tensor_tensor(cv1n[h][:, csl], cv[0:64, :], rbs[:], ALU.mult)
            else:
                cv2n = pnorm.tile([64, 1024], dt.float32, tag="cv2n")
                nc.vector.tensor_tensor(cv2n[:], cv[0:64, :], rbs[:], ALU.mult)
                nc.vector.tensor_tensor(
                    diffT[ts_h(h), csl], cv1n[h][:, csl], cv2n[:], ALU.subtract
                )

        def emit_norm(stage, tci, h, cv, tail=False):
            rbs = emit_norm_head(stage, tci, h, cv)
            if tail:
                emit_norm_tail(stage, tci, h, cv, rbs)
            else:
                # defer the mult/sub so the next tci's DVE exps are not
                # queued behind them; flushed at the next tci's blk==2
                deferred_norms.append((stage, tci, h, cv, rbs))

        # ---- output projection (2 token-blocks per psum tile) ----
        def emit_proj_out(tci, p):
            pdt = psU.tile([128, 1024], dt.float32, tag="pu")
            tb0, tb1 = 8 * tci + 2 * p, 8 * tci + 2 * p + 1
            mm_ksplit_pair(
                [pdt[:, ts(0, 512)], pdt[:, ts(1, 512)]],
                diffT[:, ts(tb0, 128)],
                [projT_s[:], projT_s[:]],
                start=True, stop=True,
                lhsT2=diffT[:, ts(tb1, 128)],
            )
            o = pout.tile([128, 1024], dt.float32, tag="po")
            if p % 2 == 0:
                nc.vector.tensor_copy(o[:], pdt[:])
            else:
                nc.scalar.activation(o[:], pdt[:], AF.Identity)
            for i in range(2):
                tb = 8 * tci + 2 * p + i
                q = nc.scalar if (tci == 1 and i == 1) else nc.sync
                q.dma_start(out_d[ts(tb, 128), :], o[:, ts(i, 512)])

        proj_work = {
            ("C", 1, 3): lambda: emit_proj_out(0, 0),
            ("C", 1, 6): lambda: emit_proj_out(0, 1),
            ("C", 1, 9): lambda: emit_proj_out(0, 2),
            ("C", 1, 12): lambda: emit_proj_out(0, 3),
        }

        # ---- attention stage (shared B/C) ----
        def exp_emit(stage, tci, blk, h, e, u):
            if _use_dve_exp(stage, tci, blk, h):
                nc.vector.tensor_scalar(
                    e[:].bitcast(dt.int16), u[:], _SCH_B / _SCH_A, _SCH_A,
                    ALU.add, ALU.mult,
                )
            else:
                nc.scalar.activation(e[:], u[:], AF.Exp)

        def attn_stage(stage, lhs_sb, nblk, aug, cv_tiles):
            for tci in range(2):
                with nc.named_scope(f"{stage}{tci}"):
                    pending = []
                    for blk in range(nblk):
                        us, es = [], []
                        for h in range(2):
                            u = psU.tile([128, 1024], dt.float32, tag="pu")
                            us.append(u)
                        for j in range(2):
                            for h in range(2):
                                hh = ts_h(h)
                                nc.tensor.matmul(
                                    us[h][:, ts(j, 512)],
                                    lhs_sb[hh, ts(blk, 128)],
                                    qx[hh, tci * 1024 + j * 512 : tci * 1024 + (j + 1) * 512],
                                    start=True,
                                    stop=True,
                                )
                        for h in range(2):
                            e = pE.tile([128, 1024], dt.bfloat16, tag="e")
                            exp_emit(stage, tci, blk, h, e, us[h])
                            if use_mask and stage == "C":
                                mk = pmk.tile([128, 1024], dt.bfloat16, tag="mk")
                                nc.sync.dma_start(
                                    mk[:],
                                    mk_d[ts(blk, 128), tci * 1024 : (tci + 1) * 1024],
                                )
                                nc.vector.tensor_tensor(e[:], e[:], mk[:], ALU.mult)
                            es.append(e)
                        pending.append((blk, es))
                        if blk == 1:
                            # must precede the first cval matmul of this tci:
                            # the tails read the previous tci's cv psum
                            while deferred_norms:
                                emit_norm_tail(*deferred_norms.pop(0))
                        if len(pending) > 2:
                            emit_cv(pending.pop(0))
                        w = a_work.pop((stage, tci, blk), None)
                        if w is not None:
                            w()
                        w = proj_work.pop((stage, tci, blk), None)
                        if w is not None:
                            w()
                    for pend in pending:
                        emit_cv(pend)
                    for h in range(2):
                        emit_norm(stage, tci, h, cv_tiles[h],
                                  tail=(stage == "C" and tci == 1))
                    if stage == "C" and tci == 1:
                        while deferred_norms:
                            emit_norm_tail(*deferred_norms.pop(0))

        def make_emit_cv(stage, aug, nblk, cv_tiles):
            def emit_cv(pend):
                blk, es = pend
                for h in range(2):
                    mm_ksplit_pair(
                        [cv_tiles[h][:, ts(0, 512)], cv_tiles[h][:, ts(1, 512)]],
                        aug[:, 130 * blk + 65 * h : 130 * blk + 65 * h + 65],
                        [es[h][:, ts(0, 512)], es[h][:, ts(1, 512)]],
                        start=(blk == 0),
                        stop=(blk == nblk - 1),
                    )
            return emit_cv

        cvB = [psCV.tile([65, 1024], dt.float32, tag=f"cv{h}", name=f"cv{h}") for h in range(2)]
        emit_cv = make_emit_cv("B", vy_aug, NMB, cvB)
        attn_stage("B", ky, NMB, vy_aug, cvB)

        cvC = [psCV.tile([65, 1024], dt.float32, tag=f"cv{h}", name=f"cv{h}") for h in range(2)]
        emit_cv = make_emit_cv("C", vx_aug, NSB, cvC)
        attn_stage("C", wxT, NSB, vx_aug, cvC)

        # tail: tci=1 output projection
        with nc.named_scope("proj1"):
            for p in range(4):
                emit_proj_out(1, p)

        if debug:
            for nm, t in [
                ("d_qx", qx), ("d_ky", ky), ("d_kx", kx),
                ("d_vy_aug", vy_aug), ("d_vx_aug", vx_aug),
                ("d_qy_tok", qy_tok), ("d_ky_tok", ky_tok), ("d_gt", gt),
                ("d_wxT", wxT), ("d_cv1n0", cv1n[0]), ("d_cv1n1", cv1n[1]),
                ("d_diffT", diffT),
            ]:
                nc.sync.dma_start(dbg[nm][:], t[:])

        psCV_cm.__exit__(None, None, None)
        psU_cm.__exit__(None, None, None)
        if use_mask:
            pmk_cm.__exit__(None, None, None)
        pout_cm.__exit__(None, None, None)
        pnorm_cm.__exit__(None, None, None)
        pE_cm.__exit__(None, None, None)
        pbig_cm.__exit__(None, None, None)
        pconst_cm.__exit__(None, None, None)

    nc.compile()
    return nc


def _get_kernel(use_mask, debug=False):
    key = (use_mask, debug)
    if key not in _kernels:
        _kernels[key] = _build(use_mask, debug)
    return _kernels[key]


def _shard_inputs(x, y, attn_x_mask, qkv_w, qkv_b, proj_w, use_mask):
    import ml_dtypes

    bf16 = ml_dtypes.bfloat16
    in_maps = []
    mask01T = None
    if use_mask:
        mask01T = np.ascontiguousarray(
            np.asarray(attn_x_mask)[0, 0].T.astype(bf16)
        )
    xT_b = [np.ascontiguousarray(x[b].T.astype(bf16)) for b in range(B)]
    yT_b = [np.ascontiguousarray(y[b].T.astype(bf16)) for b in range(B)]
    for core in range(NC):
        b, hp = divmod(core, 4)
        h0, h1 = 2 * hp, 2 * hp + 1
        hs = np.r_[h0 * D : (h0 + 1) * D, h1 * D : (h1 + 1) * D]
        w_sel = qkv_w[np.r_[hs, C + hs, 2 * C + hs], :].copy()
        w_sel[0:128, :] *= 0.125  # fold 1/sqrt(D) into q weights
        m = {
            "xT": xT_b[b],
            "yT": yT_b[b],
            "wT": np.ascontiguousarray(w_sel.T.astype(bf16)),
            "biases": np.ascontiguousarray(
                np.stack([qkv_b[hs] * 0.125, qkv_b[C + hs]]).reshape(2, 128, 1)
            ).astype(np.float32),
            "projT": np.ascontiguousarray(proj_w.T[hs, :].astype(bf16)),
        }
        if use_mask:
            m["mask01T"] = mask01T
        in_maps.append(m)
    return in_maps


def _run(x, y, attn_x_mask, qkv_w, qkv_b, proj_w, proj_b, profile=False, debug=False):
    from concourse.bass_utils import run_bass_kernel_spmd

    x = np.asarray(x, np.float32)
    y = np.asarray(y, np.float32)
    qkv_w = np.asarray(qkv_w, np.float32)
    qkv_b = np.asarray(qkv_b, np.float32)
    proj_w = np.asarray(proj_w, np.float32)
    proj_b = np.asarray(proj_b, np.float32)
    mask = np.asarray(attn_x_mask)
    use_mask = not bool(mask.all())

    if profile:
        _install_ntff_hook()
    nc = _get_kernel(use_mask, debug)
    in_maps = _shard_inputs(x, y, mask, qkv_w, qkv_b, proj_w, use_mask)
    res = run_bass_kernel_spmd(nc, in_maps, list(range(NC)), trace=profile)

    out = np.zeros((B, T, C), np.float64)
    for core in range(NC):
        b = core // 4
        out[b] += res.results[core]["out_partial"].astype(np.float64)
    out += proj_b.astype(np.float64)
    return out.astype(np.float32), res


def kernel(x, y, attn_x_mask, qkv_w, qkv_b, proj_w, proj_b):
    out, _ = _run(x, y, attn_x_mask, qkv_w, qkv_b, proj_w, proj_b, profile=False)
    return out


def kernel_profiled(x, y, attn_x_mask, qkv_w, qkv_b, proj_w, proj_b):
    out, res = _run(x, y, attn_x_mask, qkv_w, qkv_b, proj_w, proj_b, profile=True)
    return out, res
